# revision 25
# baseline (speedup 1.0000x reference)
"""GQA attention layer (B=2, S=2048, H=4096, 32 Q heads / 8 KV heads, HD=128)
on 8 trn2 NeuronCores.

Sharding: 2D = data-parallel over batch (2) x tensor-parallel over heads (4).
Core c -> (batch = c // 4, tp = c % 4): 8 Q heads, 2 KV heads, full sequence.
Wq/Wk/Wv split along output rows, Wo along input cols (Megatron TP); the
4 per-batch partial outputs are summed on the host (the TP unshard step).

Per-core kernel phases (all matmuls contract along the SBUF partition dim,
all matmul operands bf16, fp32 PSUM accumulation):
  A: K/V projections from x^T (bf16), RoPE on K          -> KTr, Vb (SBUF)
  B: Q projection + RoPE                                 -> QTr (SBUF, bf16)
  C: per (q-chunk, head), causal skip at k-tile granularity:
     scores^T = K^T-tiles x Q into 1024-wide PSUM pairs, exp on ACT per
     pair, causal masking of the 4 diagonal k-tiles via precomputed 0/1
     patterns (DVE mult, post-exp), attn @ V accumulated in PSUM,
     softmax denominator via DVE chain-adds of exp pairs + one ones-matmul,
     normalize with reciprocal broadcast by a K=1 matmul  -> ctx (SBUF, bf16)
  D: out = ctx^T x Wo^T (bf16, fp32 accum)               -> out (DRAM)

RoPE runs in the natural interleaved head layout: pair (x[2i], x[2i+1]) sits
at adjacent partitions, the partner is fetched with a swap-adjacent
stream_shuffle, and the sign/cos/sin tables are pre-interleaved on the host:
  rot = x * cc + shuffle(x * ss),  cc[2i]=cc[2i+1]=cos_i,
  ss[2i]=+sin_i, ss[2i+1]=-sin_i.
"""

import math

import numpy as np
import ml_dtypes

import concourse.bass as bass
import concourse.mybir as mybir
import concourse.tile as tile
from concourse import bacc
from concourse import bass_utils
from concourse.bass_interp import get_hw_module

B, S, H, NH, NKV, HD = 2, 2048, 4096, 32, 8, 128
TP = 4  # head-parallel cores per batch
N_CORES = 8
QH = NH // TP          # 8 q heads per core
KVH = NKV // TP        # 2 kv heads per core
QROWS = QH * HD        # 1024
KVROWS = KVH * HD      # 256
HT = H // 128          # 32 h (contraction) tiles
ST = S // 128          # 16 seq tiles
F32 = mybir.dt.float32
F32R = mybir.dt.float32r
BF16 = mybir.dt.bfloat16
AX = mybir.AluOpType
EXP = mybir.ActivationFunctionType.Exp
SWAP_ADJ = [i ^ 1 for i in range(32)]

QCH = 512              # q-chunk width in phase C
NQQ = S // QCH
CHUNK_ORDER = [2, 3, 0, 1]   # B re-uses A's still-resident x chunks 2,3 first


def build_nc(use_mask: bool, debug: bool = False):
    nc = bacc.Bacc("TRN2", target_bir_lowering=False, debug=False, num_devices=N_CORES)
    xtb = nc.dram_tensor("xtb", [H, S], BF16, kind="ExternalInput").ap()
    wqt = nc.dram_tensor("wqt", [H, QROWS], BF16, kind="ExternalInput").ap()
    wkt = nc.dram_tensor("wkt", [H, KVROWS], BF16, kind="ExternalInput").ap()
    wvt = nc.dram_tensor("wvt", [H, KVROWS], BF16, kind="ExternalInput").ap()
    wot = nc.dram_tensor("wot", [QROWS, H], BF16, kind="ExternalInput").ap()
    cs = nc.dram_tensor("cs", [128, S], F32, kind="ExternalInput").ap()
    sc = nc.dram_tensor("sc", [128, S], F32, kind="ExternalInput").ap()
    diagt = None
    if use_mask:
        # 0/1 post-exp masks for the 4 diagonal k-tiles of each q-chunk:
        # diagt[p, j, i*512 + c] = (c >= (2j+i)*128 + p)
        diagt = nc.dram_tensor("diagt", [128, 2 * 1024], BF16, kind="ExternalInput").ap()
    out = nc.dram_tensor("out", [S, H], F32, kind="ExternalOutput").ap()
    dbg = {}
    if debug:
        dbg["k"] = nc.dram_tensor("dbg_k", [128, KVH * S], BF16, kind="ExternalOutput").ap()
        dbg["q"] = nc.dram_tensor("dbg_q", [128, QH * S], BF16, kind="ExternalOutput").ap()
        dbg["v"] = nc.dram_tensor("dbg_v", [128, KVH * ST * HD], BF16, kind="ExternalOutput").ap()
        dbg["ctx"] = nc.dram_tensor("dbg_ctx", [128, QH * S], BF16, kind="ExternalOutput").ap()

    with tile.TileContext(nc) as tc:
        with tc.tile_pool(name="persist", bufs=1) as pp:
            ktr = pp.tile([128, KVH, S], BF16)          # roped K^T (1 MB)
            vb = pp.tile([128, KVH, ST, HD], BF16)      # V [seq, hd] tiles (1 MB)
            qtr = pp.tile([128, QH, S], BF16)           # roped Q^T (4 MB)
            ones_f = pp.tile([128, 1], F32)
            ones_bf = pp.tile([128, 1], BF16)
            nc.gpsimd.memset(ones_f[:], 1.0)
            nc.vector.tensor_copy(ones_bf[:], ones_f[:])

            # Shared x^T chunk pool: phase A loads chunks 0-3, phase B reuses
            # the still-resident chunks 2,3 (buffer cycling), re-loads 0,1.
            xsrc = xtb[:].rearrange("(ht p) s -> p ht s", p=128)
            with (
                tc.tile_pool(name="tables", bufs=1) as tbp,
                tc.tile_pool(name="xsh", bufs=2) as xsh,
            ):
                cs_sb = tbp.tile([128, S], F32)
                sc_sb = tbp.tile([128, S], F32)
                nc.sync.dma_start(cs_sb[:], cs[:])
                nc.sync.dma_start(sc_sb[:], sc[:])
                # ------------ Phase A: K/V projection + K rope ----------
                with (
                    tc.tile_pool(name="wkv", bufs=1) as wkvp,
                    tc.tile_pool(name="xs0", bufs=4) as xs0p,
                    tc.tile_pool(name="ropea", bufs=2) as rpa,
                    tc.tile_pool(name="psa", bufs=2, space="PSUM") as psa,
                    tc.tile_pool(name="psav", bufs=1, space="PSUM") as psav,
                ):
                    wk_sb = wkvp.tile([128, HT, KVROWS], BF16)
                    wv_sb = wkvp.tile([128, HT, KVROWS], BF16)
                    # chunk 0 in 4 sub-chunks so the first matmuls start early
                    XSUB = HT // 4
                    xa0 = []
                    for i in range(4):
                        t = xs0p.tile([128, XSUB, 512], BF16, name="xa0")
                        nc.scalar.dma_start(
                            t[:], xsrc[:, i * XSUB:(i + 1) * XSUB, 0:512])
                        if i == 0:
                            nc.sync.dma_start(wk_sb[:, 0, :], wkt[0:128, :])
                            nc.sync.dma_start(wv_sb[:, 0, :], wvt[0:128, :])
                        xa0.append(t)
                    xchunks = {}
                    for q4 in (1, 2):        # prefetch next chunks
                        xchunks[q4] = xsh.tile([128, HT, 512], BF16, name="xc")
                        nc.scalar.dma_start(
                            xchunks[q4][:], xsrc[:, :, q4 * 512:(q4 + 1) * 512])

                    def xa_(q4, h):
                        if q4 == 0:
                            return xa0[h // XSUB][:, h % XSUB, :]
                        return xchunks[q4][:, h, :]

                    for q4 in range(4):      # seq quarters of 512
                        sl = slice(q4 * 512, (q4 + 1) * 512)
                        if q4 == 1:
                            xchunks[3] = xsh.tile([128, HT, 512], BF16, name="xc")
                            nc.scalar.dma_start(
                                xchunks[3][:], xsrc[:, :, 3 * 512:4 * 512])
                        kps = psa.tile([128, KVH, 512], F32, name="kps")
                        vps = [psav.tile([128, KVROWS], F32, name=f"vps{st}")
                               for st in range(4)]
                        # all K matmuls first, then all V: the next chunk's K
                        # work covers the previous chunk's V-evict latency
                        for h in range(HT):
                            if q4 == 0 and h > 0:
                                nc.sync.dma_start(wk_sb[:, h, :],
                                                  wkt[h * 128:(h + 1) * 128, :])
                                nc.sync.dma_start(wv_sb[:, h, :],
                                                  wvt[h * 128:(h + 1) * 128, :])
                            for r in range(KVH):
                                nc.tensor.matmul(kps[:, r, :],
                                                 wk_sb[:, h, r * 128:(r + 1) * 128],
                                                 xa_(q4, h),
                                                 start=(h == 0), stop=(h == HT - 1))
                        for h in range(HT):
                            xah = xa_(q4, h)
                            for st in range(4):
                                nc.tensor.matmul(vps[st][:],
                                                 xah[:, st * 128:(st + 1) * 128],
                                                 wv_sb[:, h, :],
                                                 start=(h == 0), stop=(h == HT - 1))
                        # rope K -> ktr (bf16): rot = x*cc + shuffle(x*ss)
                        for r in range(KVH):
                            t1 = rpa.tile([128, 512], F32, name="t1")
                            m0 = rpa.tile([128, 512], F32, name="m0")
                            sw = rpa.tile([128, 512], F32, name="sw")
                            nc.vector.tensor_tensor(t1[:], kps[:, r, :], cs_sb[:, sl], op=AX.mult)
                            nc.vector.tensor_tensor(m0[:], kps[:, r, :], sc_sb[:, sl], op=AX.mult)
                            nc.vector.stream_shuffle(sw[:], m0[:], mask=SWAP_ADJ)
                            nc.vector.tensor_tensor(ktr[:, r, sl], t1[:], sw[:], op=AX.add)
                        # evict V -> vb (bf16)
                        for st in range(4):
                            nc.scalar.copy(
                                vb[:, :, q4 * 4 + st, :],
                                vps[st][:].rearrange("p (kv d) -> p kv d", kv=KVH))

                if debug:
                    nc.sync.dma_start(dbg["k"][:], ktr[:].rearrange("p kv s -> p (kv s)"))
                    nc.sync.dma_start(dbg["v"][:], vb[:].rearrange("p kv st d -> p (kv st d)"))

                # ------------ Phase B: Q projection + rope --------------
                with (
                    tc.tile_pool(name="wq", bufs=1) as wqp,
                    tc.tile_pool(name="ropeb", bufs=1) as rpb,
                    tc.tile_pool(name="psb", bufs=2, space="PSUM") as psb,
                ):
                    wq_sb = wqp.tile([128, HT, QROWS], BF16)
                    for qidx, qc in enumerate(CHUNK_ORDER):
                        sl = slice(qc * 512, (qc + 1) * 512)
                        if qc in (0, 1):     # chunks 2,3 still resident from A
                            xchunks[qc] = xsh.tile([128, HT, 512], BF16,
                                                   name="xc")
                            nc.scalar.dma_start(xchunks[qc][:],
                                              xsrc[:, :, qc * 512:(qc + 1) * 512])
                        xb = xchunks[qc]
                        for rh in range(2):  # row halves of 512 (4 heads each)
                            rsl = slice(rh * 512, (rh + 1) * 512)
                            qps = psb.tile([128, 4, 512], F32, name="qps")
                            for h in range(HT):
                                if qidx == 0:
                                    nc.sync.dma_start(wq_sb[:, h, rsl],
                                                      wqt[h * 128:(h + 1) * 128, rsl])
                                for r in range(4):
                                    nc.tensor.matmul(
                                        qps[:, r, :],
                                        wq_sb[:, h, rh * 512 + r * 128:rh * 512 + (r + 1) * 128],
                                        xb[:, h, :],
                                        start=(h == 0), stop=(h == HT - 1))
                            for r in range(4):
                                head = rh * 4 + r
                                t1 = rpb.tile([128, 512], F32, name="t1")
                                m0 = rpb.tile([128, 512], F32, name="m0")
                                sw = rpb.tile([128, 512], F32, name="sw")
                                nc.vector.tensor_tensor(t1[:], qps[:, r, :], cs_sb[:, sl], op=AX.mult)
                                nc.vector.tensor_tensor(m0[:], qps[:, r, :], sc_sb[:, sl], op=AX.mult)
                                nc.vector.stream_shuffle(sw[:], m0[:], mask=SWAP_ADJ)
                                nc.vector.tensor_tensor(qtr[:, head, sl], t1[:], sw[:], op=AX.add)

            # ---------------- Phase C: attention ------------------------
            with (
                tc.tile_pool(name="ctxp", bufs=1) as ctxp,
                tc.tile_pool(name="wo", bufs=1) as wop,
            ):
                ctx_sb = ctxp.tile([128, QH, S], BF16)
                wo_sb = wop.tile([128, QH, H], BF16)   # prefetched during C
                with (
                    tc.tile_pool(name="diagp", bufs=1) as dgp,
                    tc.tile_pool(name="expws", bufs=1) as expp,
                    tc.tile_pool(name="dsump", bufs=2) as dsp,
                    tc.tile_pool(name="smallc", bufs=2) as smc,
                    tc.tile_pool(name="ob", bufs=4) as obp,
                    tc.tile_pool(name="pscs", bufs=2, space="PSUM") as pscs,
                    tc.tile_pool(name="pscx", bufs=2, space="PSUM") as pscx,
                    tc.tile_pool(name="pscb", bufs=1, space="PSUM") as pscb,
                    tc.tile_pool(name="psd", bufs=1, space="PSUM") as psd,
                ):
                    diag = None
                    if use_mask:
                        diag = dgp.tile([128, 2, 1024], BF16)
                        nc.sync.dma_start(
                            diag[:], diagt[:].rearrange("p (j w) -> p j w", j=2))
                    for h in range(QH):    # wo prefetch, after diag on the queue
                        nc.sync.dma_start(wo_sb[:, h, :],
                                          wot[h * 128:(h + 1) * 128, :])
                    exp_ws = expp.tile([128, ST // 2, 1024], BF16)

                    def d_unit(qq_d, u):
                        # phase-D slice: one 512-col group of one seq tile of
                        # a finished chunk, interleaved into C's pipeline
                        st = qq_d * 4 + u // 8
                        g = u % 8
                        ops = psd.tile([128, 512], F32, name="ops")
                        for hh in range(QH):
                            nc.tensor.matmul(
                                ops[:],
                                ctx_sb[:, hh, st * 128:(st + 1) * 128],
                                wo_sb[:, hh, g * 512:(g + 1) * 512],
                                start=(hh == 0), stop=(hh == QH - 1))
                        osb = obp.tile([128, 512], F32, name="osb")
                        nc.vector.tensor_copy(osb[:], ops[:])
                        nc.sync.dma_start(
                            out[st * 128:(st + 1) * 128, g * 512:(g + 1) * 512],
                            osb[:])

                    prev_d = None
                    for qq in CHUNK_ORDER:
                        qsl = slice(qq * QCH, (qq + 1) * QCH)
                        L = 4 * (qq + 1) if use_mask else ST   # live k tiles
                        P = L // 2                             # 1024-wide pairs
                        # software-pipelined over heads: scores/exp/AV for h,
                        # denominator finish + normalize for h-1
                        prev = None
                        for h in range(QH + 1):
                            if h < QH:
                                kvh = h // (QH // KVH)
                                ctxps = pscx.tile([128, QCH], F32, name="ctxps")
                                dsum = dsp.tile([128, 1024], BF16, name="dsum")
                                # AV + denominator lag scores/exp by two pairs
                                # so PE never waits on the ACT exp
                                for p in range(P + 2):
                                    if p < P:
                                        sps = pscs.tile([128, 1024], F32, name="sps")
                                        for i in range(2):
                                            kt = 2 * p + i
                                            nc.tensor.matmul(
                                                sps[:, i * 512:(i + 1) * 512],
                                                ktr[:, kvh, kt * 128:(kt + 1) * 128],
                                                qtr[:, h, qsl],
                                                start=True, stop=True)
                                        nc.scalar.activation(exp_ws[:, p, :], sps[:], EXP)
                                        if use_mask and p >= P - 2:
                                            nc.vector.tensor_tensor(
                                                exp_ws[:, p, :], exp_ws[:, p, :],
                                                diag[:, p - (P - 2), :], op=AX.mult)
                                    if 2 <= p:
                                        pp_ = p - 2
                                        if pp_ < P:
                                            for i in range(2):
                                                kt = 2 * pp_ + i
                                                nc.tensor.matmul(
                                                    ctxps[:],
                                                    vb[:, kvh, kt, :],
                                                    exp_ws[:, pp_, i * 512:(i + 1) * 512],
                                                    start=(kt == 0), stop=(kt == L - 1))
                                            if pp_ == 0:
                                                nc.vector.tensor_copy(dsum[:], exp_ws[:, 0, :])
                                            else:
                                                nc.vector.tensor_tensor(
                                                    dsum[:], dsum[:], exp_ws[:, pp_, :],
                                                    op=AX.add)
                            if h > 0:
                                # finish previous head's softmax denominator
                                p_ctxps, p_dsum, p_qsl = prev
                                fold = smc.tile([128, 512], BF16, name="fold")
                                nc.vector.tensor_tensor(
                                    fold[:], p_dsum[:, 0:512], p_dsum[:, 512:1024],
                                    op=AX.add)
                                dps_t = pscb.tile([128, 512], F32, name="dps")
                                dps = dps_t[0:1, :]
                                nc.tensor.matmul(dps, ones_bf[:], fold[:],
                                                 start=True, stop=True)
                                rf = smc.tile([1, 512], F32, name="rf")
                                nc.vector.reciprocal_approx_fast(rf[:], dps)
                                # broadcast 1/denom to all partitions on gpsimd
                                rbbf = smc.tile([128, 512], F32, name="rbbf")
                                nc.gpsimd.partition_broadcast(rbbf[:], rf[:])
                                nc.vector.tensor_tensor(
                                    ctx_sb[:, h - 1, p_qsl], p_ctxps[:], rbbf[:],
                                    op=AX.mult)
                            if prev_d is not None and h < QH:
                                for u in range(4 * h, 4 * h + 4):
                                    d_unit(prev_d, u)
                            if h < QH:
                                prev = (ctxps, dsum, qsl)
                        prev_d = qq
                    # last chunk's output projection, un-interleaved
                    for u in range(32):
                        d_unit(prev_d, u)

                if debug:
                    nc.sync.dma_start(dbg["q"][:], qtr[:].rearrange("p h s -> p (h s)"))
                    nc.sync.dma_start(dbg["ctx"][:], ctx_sb[:].rearrange("p h s -> p (h s)"))

    nc.compile()
    nc.m = get_hw_module(nc.m)
    return nc


_NC_CACHE = {}


def _get_nc(use_mask: bool, debug: bool = False):
    key = (use_mask, debug)
    if key not in _NC_CACHE:
        _NC_CACHE[key] = build_nc(use_mask, debug)
    return _NC_CACHE[key]


def _diag_patterns():
    # diagt[p, j*1024 + i*512 + c] = 1.0 if c >= (2j+i)*128 + p else 0.0
    p = np.arange(128)[:, None]
    c = np.arange(512)[None, :]
    blocks = [(c >= kt * 128 + p).astype(np.float32) for kt in range(4)]
    return np.concatenate(blocks, axis=1).astype(ml_dtypes.bfloat16)  # [128, 2048]


def kernel(hidden_states, cos, sin, position_ids, attention_mask, Wq, Wk, Wv, Wo,
           _trace=False, _debug=False):
    hidden_states = np.asarray(hidden_states, np.float32)
    cos = np.asarray(cos, np.float32)
    sin = np.asarray(sin, np.float32)
    position_ids = np.asarray(position_ids)
    attention_mask = np.asarray(attention_mask, np.float32)
    Wq = np.asarray(Wq, np.float32)
    Wk = np.asarray(Wk, np.float32)
    Wv = np.asarray(Wv, np.float32)
    Wo = np.asarray(Wo, np.float32)

    use_mask = bool(np.any(attention_mask))
    if use_mask:
        # the fast path hardcodes causal structure; verify it applies
        causal = np.where(
            np.tril(np.ones((S, S), bool)), 0.0, np.finfo(np.float32).min
        ).astype(np.float32)
        assert np.array_equal(
            attention_mask, np.broadcast_to(causal, (B, 1, S, S))
        ), "only the causal (or all-zeros) attention mask is supported"
    nc = _get_nc(use_mask, _debug)

    scale = 1.0 / math.sqrt(HD)
    wqt_full = np.ascontiguousarray((Wq * scale).T).astype(ml_dtypes.bfloat16)
    wkt_full = np.ascontiguousarray(Wk.T).astype(ml_dtypes.bfloat16)
    wvt_full = np.ascontiguousarray(Wv.T).astype(ml_dtypes.bfloat16)
    wot_full = np.ascontiguousarray(Wo.T).astype(ml_dtypes.bfloat16)
    diagt = _diag_patterns() if use_mask else None

    pos = np.asarray(position_ids, np.int64)
    in_maps = []
    per_batch = {}
    for b in range(B):
        xtb = np.ascontiguousarray(hidden_states[b].T).astype(ml_dtypes.bfloat16)
        cg = cos[pos[b]]                                     # [2048, 64]
        sg = sin[pos[b]]
        cs_b = np.repeat(cg.T, 2, axis=0).astype(np.float32)         # cc
        sc_b = np.empty((HD, S), np.float32)                         # ss
        sc_b[0::2] = sg.T
        sc_b[1::2] = -sg.T
        per_batch[b] = (xtb, cs_b, sc_b)

    for c in range(N_CORES):
        b, tp = c // TP, c % TP
        xtb, cs_b, sc_b = per_batch[b]
        m = {
            "xtb": xtb,
            "wqt": np.ascontiguousarray(wqt_full[:, tp * QROWS:(tp + 1) * QROWS]),
            "wkt": np.ascontiguousarray(wkt_full[:, tp * KVROWS:(tp + 1) * KVROWS]),
            "wvt": np.ascontiguousarray(wvt_full[:, tp * KVROWS:(tp + 1) * KVROWS]),
            "wot": np.ascontiguousarray(wot_full[tp * QROWS:(tp + 1) * QROWS, :]),
            "cs": cs_b,
            "sc": sc_b,
        }
        if use_mask:
            m["diagt"] = diagt
        in_maps.append(m)

    res = bass_utils.run_bass_kernel_spmd(
        nc, in_maps, core_ids=list(range(N_CORES)), trace=_trace)

    out = np.zeros((B, S, H), np.float32)
    for c in range(N_CORES):
        out[c // TP] += res.results[c]["out"]
    if _trace:
        kernel._last_results = res
    return out


# revision 29
# speedup vs baseline: 1.0323x; 1.0323x over previous
"""GQA attention layer (B=2, S=2048, H=4096, 32 Q heads / 8 KV heads, HD=128)
on 8 trn2 NeuronCores.

Sharding: 2D = data-parallel over batch (2) x tensor-parallel over heads (4).
Core c -> (batch = c // 4, tp = c % 4): 8 Q heads, 2 KV heads, full sequence.
Wq/Wk/Wv split along output rows, Wo along input cols (Megatron TP); the
4 per-batch partial outputs are summed on the host (the TP unshard step).

Per-core kernel phases (all matmuls contract along the SBUF partition dim,
all matmul operands bf16, fp32 PSUM accumulation):
  A: K/V projections from x^T (bf16), RoPE on K          -> KTr, Vb (SBUF)
  B: Q projection + RoPE                                 -> QTr (SBUF, bf16)
  C: per (q-chunk, head), causal skip at k-tile granularity:
     scores^T = K^T-tiles x Q into 1024-wide PSUM pairs, exp on ACT per
     pair, causal masking of the 4 diagonal k-tiles via precomputed 0/1
     patterns (DVE mult, post-exp), attn @ V accumulated in PSUM,
     softmax denominator via DVE chain-adds of exp pairs + one ones-matmul,
     normalize with reciprocal broadcast by a K=1 matmul  -> ctx (SBUF, bf16)
  D: out = ctx^T x Wo^T (bf16, fp32 accum)               -> out (DRAM)

RoPE runs in the natural interleaved head layout: pair (x[2i], x[2i+1]) sits
at adjacent partitions, the partner is fetched with a swap-adjacent
stream_shuffle, and the sign/cos/sin tables are pre-interleaved on the host:
  rot = x * cc + shuffle(x * ss),  cc[2i]=cc[2i+1]=cos_i,
  ss[2i]=+sin_i, ss[2i+1]=-sin_i.
"""

import math

import numpy as np
import ml_dtypes

import concourse.bass as bass
import concourse.mybir as mybir
import concourse.tile as tile
from concourse import bacc
from concourse import bass_utils
from concourse.bass_interp import get_hw_module

B, S, H, NH, NKV, HD = 2, 2048, 4096, 32, 8, 128
TP = 4  # head-parallel cores per batch
N_CORES = 8
QH = NH // TP          # 8 q heads per core
KVH = NKV // TP        # 2 kv heads per core
QROWS = QH * HD        # 1024
KVROWS = KVH * HD      # 256
HT = H // 128          # 32 h (contraction) tiles
ST = S // 128          # 16 seq tiles
F32 = mybir.dt.float32
F32R = mybir.dt.float32r
BF16 = mybir.dt.bfloat16
AX = mybir.AluOpType
EXP = mybir.ActivationFunctionType.Exp
SWAP_ADJ = [i ^ 1 for i in range(32)]

QCH = 512              # q-chunk width in phase C
NQQ = S // QCH
CHUNK_ORDER = [2, 3, 0, 1]   # B re-uses A's still-resident x chunks 2,3 first


def build_nc(use_mask: bool, debug: bool = False):
    nc = bacc.Bacc("TRN2", target_bir_lowering=False, debug=False, num_devices=N_CORES)
    xtb = nc.dram_tensor("xtb", [H, S], BF16, kind="ExternalInput").ap()
    wqt = nc.dram_tensor("wqt", [H, QROWS], BF16, kind="ExternalInput").ap()
    wkt = nc.dram_tensor("wkt", [H, KVROWS], BF16, kind="ExternalInput").ap()
    wvt = nc.dram_tensor("wvt", [H, KVROWS], BF16, kind="ExternalInput").ap()
    wot = nc.dram_tensor("wot", [QROWS, H], BF16, kind="ExternalInput").ap()
    cs = nc.dram_tensor("cs", [128, S], F32, kind="ExternalInput").ap()
    sc = nc.dram_tensor("sc", [128, S], F32, kind="ExternalInput").ap()
    diagt = None
    if use_mask:
        # 0/1 post-exp masks for the 4 diagonal k-tiles of each q-chunk:
        # diagt[p, j, i*512 + c] = (c >= (2j+i)*128 + p)
        diagt = nc.dram_tensor("diagt", [128, 2 * 1024], BF16, kind="ExternalInput").ap()
    out = nc.dram_tensor("out", [S, H], F32, kind="ExternalOutput").ap()
    dbg = {}
    if debug:
        dbg["k"] = nc.dram_tensor("dbg_k", [128, KVH * S], BF16, kind="ExternalOutput").ap()
        dbg["q"] = nc.dram_tensor("dbg_q", [128, QH * S], BF16, kind="ExternalOutput").ap()
        dbg["v"] = nc.dram_tensor("dbg_v", [128, KVH * ST * HD], BF16, kind="ExternalOutput").ap()
        dbg["ctx"] = nc.dram_tensor("dbg_ctx", [128, QH * S], BF16, kind="ExternalOutput").ap()

    with tile.TileContext(nc) as tc:
        with tc.tile_pool(name="persist", bufs=1) as pp:
            ktr = pp.tile([128, KVH, S], BF16)          # roped K^T (1 MB)
            vb = pp.tile([128, KVH, ST, HD], BF16)      # V [seq, hd] tiles (1 MB)
            qtr = pp.tile([128, QH, S], BF16)           # roped Q^T (4 MB)
            ones_f = pp.tile([128, 1], F32)
            ones_bf = pp.tile([128, 1], BF16)
            nc.gpsimd.memset(ones_f[:], 1.0)
            nc.vector.tensor_copy(ones_bf[:], ones_f[:])

            # Shared x^T chunk pool: phase A loads chunks 0-3, phase B reuses
            # the still-resident chunks 2,3 (buffer cycling), re-loads 0,1.
            xsrc = xtb[:].rearrange("(ht p) s -> p ht s", p=128)
            with (
                tc.tile_pool(name="tables", bufs=1) as tbp,
                tc.tile_pool(name="xsh", bufs=2) as xsh,
            ):
                cs_sb = tbp.tile([128, S], F32)
                sc_sb = tbp.tile([128, S], F32)
                nc.sync.dma_start(cs_sb[:], cs[:])
                nc.sync.dma_start(sc_sb[:], sc[:])
                # ------------ Phase A: K/V projection + K rope ----------
                with (
                    tc.tile_pool(name="wkv", bufs=1) as wkvp,
                    tc.tile_pool(name="xs0", bufs=4) as xs0p,
                    tc.tile_pool(name="ropea", bufs=2) as rpa,
                    tc.tile_pool(name="psa", bufs=2, space="PSUM") as psa,
                    tc.tile_pool(name="psav", bufs=1, space="PSUM") as psav,
                ):
                    wk_sb = wkvp.tile([128, HT, KVROWS], BF16)
                    wv_sb = wkvp.tile([128, HT, KVROWS], BF16)
                    # chunk 0 in 4 sub-chunks so the first matmuls start early
                    XSUB = HT // 4
                    xa0 = []
                    for i in range(4):
                        t = xs0p.tile([128, XSUB, 512], BF16, name="xa0")
                        nc.scalar.dma_start(
                            t[:], xsrc[:, i * XSUB:(i + 1) * XSUB, 0:512])
                        if i == 0:
                            nc.sync.dma_start(wk_sb[:, 0, :], wkt[0:128, :])
                            nc.sync.dma_start(wv_sb[:, 0, :], wvt[0:128, :])
                        xa0.append(t)
                    xchunks = {}
                    for q4 in (1, 2):        # prefetch next chunks
                        xchunks[q4] = xsh.tile([128, HT, 512], BF16, name="xc")
                        nc.scalar.dma_start(
                            xchunks[q4][:], xsrc[:, :, q4 * 512:(q4 + 1) * 512])

                    def xa_(q4, h):
                        if q4 == 0:
                            return xa0[h // XSUB][:, h % XSUB, :]
                        return xchunks[q4][:, h, :]

                    for q4 in range(4):      # seq quarters of 512
                        sl = slice(q4 * 512, (q4 + 1) * 512)
                        if q4 == 1:
                            xchunks[3] = xsh.tile([128, HT, 512], BF16, name="xc")
                            nc.scalar.dma_start(
                                xchunks[3][:], xsrc[:, :, 3 * 512:4 * 512])
                        kps = psa.tile([128, KVH, 512], F32, name="kps")
                        vps = [psav.tile([128, KVROWS], F32, name=f"vps{st}")
                               for st in range(4)]
                        # all K matmuls first, then all V: the next chunk's K
                        # work covers the previous chunk's V-evict latency
                        for h in range(HT):
                            if q4 == 0 and h > 0:
                                nc.sync.dma_start(wk_sb[:, h, :],
                                                  wkt[h * 128:(h + 1) * 128, :])
                                nc.sync.dma_start(wv_sb[:, h, :],
                                                  wvt[h * 128:(h + 1) * 128, :])
                            for r in range(KVH):
                                nc.tensor.matmul(kps[:, r, :],
                                                 wk_sb[:, h, r * 128:(r + 1) * 128],
                                                 xa_(q4, h),
                                                 start=(h == 0), stop=(h == HT - 1))
                        for h in range(HT):
                            xah = xa_(q4, h)
                            for st in range(4):
                                nc.tensor.matmul(vps[st][:],
                                                 xah[:, st * 128:(st + 1) * 128],
                                                 wv_sb[:, h, :],
                                                 start=(h == 0), stop=(h == HT - 1))
                        # rope K -> ktr (bf16): rot = x*cc + shuffle(x*ss)
                        for r in range(KVH):
                            t1 = rpa.tile([128, 512], F32, name="t1")
                            m0 = rpa.tile([128, 512], F32, name="m0")
                            sw = rpa.tile([128, 512], F32, name="sw")
                            nc.vector.tensor_tensor(t1[:], kps[:, r, :], cs_sb[:, sl], op=AX.mult)
                            nc.vector.tensor_tensor(m0[:], kps[:, r, :], sc_sb[:, sl], op=AX.mult)
                            nc.vector.stream_shuffle(sw[:], m0[:], mask=SWAP_ADJ)
                            nc.vector.tensor_tensor(ktr[:, r, sl], t1[:], sw[:], op=AX.add)
                        # evict V -> vb (bf16)
                        for st in range(4):
                            nc.scalar.copy(
                                vb[:, :, q4 * 4 + st, :],
                                vps[st][:].rearrange("p (kv d) -> p kv d", kv=KVH))

                if debug:
                    nc.sync.dma_start(dbg["k"][:], ktr[:].rearrange("p kv s -> p (kv s)"))
                    nc.sync.dma_start(dbg["v"][:], vb[:].rearrange("p kv st d -> p (kv st d)"))

                # ------------ Phase B: Q projection + rope --------------
                with (
                    tc.tile_pool(name="wq", bufs=1) as wqp,
                    tc.tile_pool(name="ropeb", bufs=1) as rpb,
                    tc.tile_pool(name="psb", bufs=2, space="PSUM") as psb,
                ):
                    wq_sb = wqp.tile([128, HT, QROWS], BF16)
                    for qidx, qc in enumerate(CHUNK_ORDER):
                        sl = slice(qc * 512, (qc + 1) * 512)
                        if qc in (0, 1):     # chunks 2,3 still resident from A
                            xchunks[qc] = xsh.tile([128, HT, 512], BF16,
                                                   name="xc")
                            nc.scalar.dma_start(xchunks[qc][:],
                                              xsrc[:, :, qc * 512:(qc + 1) * 512])
                        xb = xchunks[qc]
                        for rh in range(2):  # row halves of 512 (4 heads each)
                            rsl = slice(rh * 512, (rh + 1) * 512)
                            qps = psb.tile([128, 4, 512], F32, name="qps")
                            for h in range(HT):
                                if qidx == 0:
                                    nc.sync.dma_start(wq_sb[:, h, rsl],
                                                      wqt[h * 128:(h + 1) * 128, rsl])
                                for r in range(4):
                                    nc.tensor.matmul(
                                        qps[:, r, :],
                                        wq_sb[:, h, rh * 512 + r * 128:rh * 512 + (r + 1) * 128],
                                        xb[:, h, :],
                                        start=(h == 0), stop=(h == HT - 1))
                            for r in range(4):
                                head = rh * 4 + r
                                t1 = rpb.tile([128, 512], F32, name="t1")
                                m0 = rpb.tile([128, 512], F32, name="m0")
                                sw = rpb.tile([128, 512], F32, name="sw")
                                nc.vector.tensor_tensor(t1[:], qps[:, r, :], cs_sb[:, sl], op=AX.mult)
                                nc.vector.tensor_tensor(m0[:], qps[:, r, :], sc_sb[:, sl], op=AX.mult)
                                nc.vector.stream_shuffle(sw[:], m0[:], mask=SWAP_ADJ)
                                nc.vector.tensor_tensor(qtr[:, head, sl], t1[:], sw[:], op=AX.add)

            # ---------------- Phase C: attention ------------------------
            with (
                tc.tile_pool(name="ctxp", bufs=1) as ctxp,
                tc.tile_pool(name="wo", bufs=1) as wop,
            ):
                ctx_sb = ctxp.tile([128, QH, S], BF16)
                wo_sb = wop.tile([128, QH, H], BF16)   # prefetched during C
                with (
                    tc.tile_pool(name="diagp", bufs=1) as dgp,
                    tc.tile_pool(name="expws", bufs=1) as expp,
                    tc.tile_pool(name="dsump", bufs=2) as dsp,
                    tc.tile_pool(name="smallc", bufs=2) as smc,
                    tc.tile_pool(name="ob", bufs=4) as obp,
                    tc.tile_pool(name="pscs", bufs=2, space="PSUM") as pscs,
                    tc.tile_pool(name="pscx", bufs=2, space="PSUM") as pscx,
                    tc.tile_pool(name="pscb", bufs=1, space="PSUM") as pscb,
                    tc.tile_pool(name="psd", bufs=1, space="PSUM") as psd,
                ):
                    diag = None
                    if use_mask:
                        diag = dgp.tile([128, 2, 1024], BF16)
                        nc.sync.dma_start(
                            diag[:], diagt[:].rearrange("p (j w) -> p j w", j=2))
                    for h in range(QH):    # wo prefetch, after diag on the queue
                        nc.sync.dma_start(wo_sb[:, h, :],
                                          wot[h * 128:(h + 1) * 128, :])
                    exp_ws = expp.tile([128, ST // 2, 1024], BF16)

                    def d_unit(qq_d, u):
                        # phase-D slice: one 512-col group of one seq tile of
                        # a finished chunk, interleaved into C's pipeline
                        st = qq_d * 4 + u // 8
                        g = u % 8
                        ops = psd.tile([128, 512], F32, name="ops")
                        for hh in range(QH):
                            nc.tensor.matmul(
                                ops[:],
                                ctx_sb[:, hh, st * 128:(st + 1) * 128],
                                wo_sb[:, hh, g * 512:(g + 1) * 512],
                                start=(hh == 0), stop=(hh == QH - 1))
                        osb = obp.tile([128, 512], F32, name="osb")
                        nc.scalar.copy(osb[:], ops[:])
                        nc.sync.dma_start(
                            out[st * 128:(st + 1) * 128, g * 512:(g + 1) * 512],
                            osb[:])

                    prev_d = None
                    for qq in CHUNK_ORDER:
                        qsl = slice(qq * QCH, (qq + 1) * QCH)
                        L = 4 * (qq + 1) if use_mask else ST   # live k tiles
                        P = L // 2                             # 1024-wide pairs
                        # software-pipelined over heads: scores/exp/AV for h,
                        # denominator finish + normalize for h-1
                        prev = None
                        for h in range(QH + 1):
                            if h < QH:
                                kvh = h // (QH // KVH)
                                ctxps = pscx.tile([128, QCH], F32, name="ctxps")
                                dsum = dsp.tile([128, 1024], BF16, name="dsum")
                                # AV + denominator lag scores/exp by two pairs
                                # so PE never waits on the ACT exp
                                if prev_d is not None:
                                    for u in range(4 * h, 4 * h + 2):
                                        d_unit(prev_d, u)
                                for p in range(P + 2):
                                    if p < P:
                                        sps = pscs.tile([128, 1024], F32, name="sps")
                                        for i in range(2):
                                            kt = 2 * p + i
                                            nc.tensor.matmul(
                                                sps[:, i * 512:(i + 1) * 512],
                                                ktr[:, kvh, kt * 128:(kt + 1) * 128],
                                                qtr[:, h, qsl],
                                                start=True, stop=True)
                                        nc.scalar.activation(exp_ws[:, p, :], sps[:], EXP)
                                        if use_mask and p >= P - 2:
                                            nc.vector.tensor_tensor(
                                                exp_ws[:, p, :], exp_ws[:, p, :],
                                                diag[:, p - (P - 2), :], op=AX.mult)
                                    if 2 <= p:
                                        pp_ = p - 2
                                        if pp_ < P:
                                            for i in range(2):
                                                kt = 2 * pp_ + i
                                                nc.tensor.matmul(
                                                    ctxps[:],
                                                    vb[:, kvh, kt, :],
                                                    exp_ws[:, pp_, i * 512:(i + 1) * 512],
                                                    start=(kt == 0), stop=(kt == L - 1))
                                            if pp_ == 0:
                                                nc.vector.tensor_copy(dsum[:], exp_ws[:, 0, :])
                                            else:
                                                nc.vector.tensor_tensor(
                                                    dsum[:], dsum[:], exp_ws[:, pp_, :],
                                                    op=AX.add)
                            if h > 0:
                                # finish previous head's softmax denominator
                                p_ctxps, p_dsum, p_qsl = prev
                                fold = smc.tile([128, 512], BF16, name="fold")
                                nc.vector.tensor_tensor(
                                    fold[:], p_dsum[:, 0:512], p_dsum[:, 512:1024],
                                    op=AX.add)
                                dps_t = pscb.tile([128, 512], F32, name="dps")
                                dps = dps_t[0:1, :]
                                nc.tensor.matmul(dps, ones_bf[:], fold[:],
                                                 start=True, stop=True)
                                rf = smc.tile([1, 512], F32, name="rf")
                                nc.vector.reciprocal_approx_fast(rf[:], dps)
                                # broadcast 1/denom to all partitions on gpsimd
                                rbbf = smc.tile([128, 512], F32, name="rbbf")
                                nc.gpsimd.partition_broadcast(rbbf[:], rf[:])
                                nc.vector.tensor_tensor(
                                    ctx_sb[:, h - 1, p_qsl], p_ctxps[:], rbbf[:],
                                    op=AX.mult)
                            if prev_d is not None and h < QH:
                                for u in range(4 * h + 2, 4 * h + 4):
                                    d_unit(prev_d, u)
                            if h < QH:
                                prev = (ctxps, dsum, qsl)
                        prev_d = qq
                    # last chunk's output projection, un-interleaved
                    for u in range(32):
                        d_unit(prev_d, u)

                if debug:
                    nc.sync.dma_start(dbg["q"][:], qtr[:].rearrange("p h s -> p (h s)"))
                    nc.sync.dma_start(dbg["ctx"][:], ctx_sb[:].rearrange("p h s -> p (h s)"))

    nc.compile()
    nc.m = get_hw_module(nc.m)
    return nc


_NC_CACHE = {}


def _get_nc(use_mask: bool, debug: bool = False):
    key = (use_mask, debug)
    if key not in _NC_CACHE:
        _NC_CACHE[key] = build_nc(use_mask, debug)
    return _NC_CACHE[key]


def _diag_patterns():
    # diagt[p, j*1024 + i*512 + c] = 1.0 if c >= (2j+i)*128 + p else 0.0
    p = np.arange(128)[:, None]
    c = np.arange(512)[None, :]
    blocks = [(c >= kt * 128 + p).astype(np.float32) for kt in range(4)]
    return np.concatenate(blocks, axis=1).astype(ml_dtypes.bfloat16)  # [128, 2048]


def kernel(hidden_states, cos, sin, position_ids, attention_mask, Wq, Wk, Wv, Wo,
           _trace=False, _debug=False):
    hidden_states = np.asarray(hidden_states, np.float32)
    cos = np.asarray(cos, np.float32)
    sin = np.asarray(sin, np.float32)
    position_ids = np.asarray(position_ids)
    attention_mask = np.asarray(attention_mask, np.float32)
    Wq = np.asarray(Wq, np.float32)
    Wk = np.asarray(Wk, np.float32)
    Wv = np.asarray(Wv, np.float32)
    Wo = np.asarray(Wo, np.float32)

    use_mask = bool(np.any(attention_mask))
    if use_mask:
        # the fast path hardcodes causal structure; verify it applies
        causal = np.where(
            np.tril(np.ones((S, S), bool)), 0.0, np.finfo(np.float32).min
        ).astype(np.float32)
        assert np.array_equal(
            attention_mask, np.broadcast_to(causal, (B, 1, S, S))
        ), "only the causal (or all-zeros) attention mask is supported"
    nc = _get_nc(use_mask, _debug)

    scale = 1.0 / math.sqrt(HD)
    wqt_full = np.ascontiguousarray((Wq * scale).T).astype(ml_dtypes.bfloat16)
    wkt_full = np.ascontiguousarray(Wk.T).astype(ml_dtypes.bfloat16)
    wvt_full = np.ascontiguousarray(Wv.T).astype(ml_dtypes.bfloat16)
    wot_full = np.ascontiguousarray(Wo.T).astype(ml_dtypes.bfloat16)
    diagt = _diag_patterns() if use_mask else None

    pos = np.asarray(position_ids, np.int64)
    in_maps = []
    per_batch = {}
    for b in range(B):
        xtb = np.ascontiguousarray(hidden_states[b].T).astype(ml_dtypes.bfloat16)
        cg = cos[pos[b]]                                     # [2048, 64]
        sg = sin[pos[b]]
        cs_b = np.repeat(cg.T, 2, axis=0).astype(np.float32)         # cc
        sc_b = np.empty((HD, S), np.float32)                         # ss
        sc_b[0::2] = sg.T
        sc_b[1::2] = -sg.T
        per_batch[b] = (xtb, cs_b, sc_b)

    for c in range(N_CORES):
        b, tp = c // TP, c % TP
        xtb, cs_b, sc_b = per_batch[b]
        m = {
            "xtb": xtb,
            "wqt": np.ascontiguousarray(wqt_full[:, tp * QROWS:(tp + 1) * QROWS]),
            "wkt": np.ascontiguousarray(wkt_full[:, tp * KVROWS:(tp + 1) * KVROWS]),
            "wvt": np.ascontiguousarray(wvt_full[:, tp * KVROWS:(tp + 1) * KVROWS]),
            "wot": np.ascontiguousarray(wot_full[tp * QROWS:(tp + 1) * QROWS, :]),
            "cs": cs_b,
            "sc": sc_b,
        }
        if use_mask:
            m["diagt"] = diagt
        in_maps.append(m)

    res = bass_utils.run_bass_kernel_spmd(
        nc, in_maps, core_ids=list(range(N_CORES)), trace=_trace)

    out = np.zeros((B, S, H), np.float32)
    for c in range(N_CORES):
        out[c // TP] += res.results[c]["out"]
    if _trace:
        kernel._last_results = res
    return out
